# revision 3
# baseline (speedup 1.0000x reference)
"""Trainium2 Bass kernel for the sparse_attention (channel-attention) module.

Computation per sample (x_s, xh_s are [512, 1152] slices):
    theta = Wt @ x_s  + bt        (fold 1/512 into Wt, bt)
    phi   = Wp @ xh_s + bp
    g     = Wg @ xh_s + bg
    att   = theta @ phi^T         (contract over n; includes the /512)
    y     = att @ g
    out   = (Ww @ y) * inv + off + x_s      (BN folded: inv into Ww, off = (bw-mean)*inv+beta)

Sharding: pure data parallel, 4 samples per core across 8 cores.

Layout: theta/phi are computed directly in transposed form thetaT[n, i]
(x blocks as the stationary matmul operand), so the attention contraction
over n needs no on-chip transposes. attT[j, i] is computed directly as
phiT^T @ thetaT, which is exactly the stationary operand the y-matmuls need.
All matmuls run in float32r (full PE rate at free-dim >= 256, ~tf32 precision).
"""

import numpy as np

import concourse.bass as bass
import concourse.mybir as mybir
from concourse import bacc
from concourse.tile import TileContext
from concourse import bass_utils

B, DIM, H, W = 32, 512, 48, 24
N = H * W            # 1152
P = 128
CB = DIM // P        # 4 channel blocks
NB = N // P          # 9 n blocks
NCH = 3
CHW = N // NCH       # 384 (>=256 keeps fp32r at full rate)
NCORES = 8
BL = B // NCORES     # 4 samples per core

_f32 = mybir.dt.float32
_f32r = mybir.dt.float32r
_add = mybir.AluOpType.add

_PROGRAM = None


def _build_program():
    nc = bacc.Bacc("TRN2", target_bir_lowering=False, debug=False)

    x4 = nc.dram_tensor("x4", [BL, CB, P, N], _f32r, kind="ExternalInput").ap()
    xh4 = nc.dram_tensor("xh4", [BL, CB, P, N], _f32r, kind="ExternalInput").ap()
    wgT = nc.dram_tensor("wgT", [CB, P, DIM], _f32r, kind="ExternalInput").ap()
    wtT = nc.dram_tensor("wtT", [CB, P, DIM], _f32r, kind="ExternalInput").ap()
    wpT = nc.dram_tensor("wpT", [CB, P, DIM], _f32r, kind="ExternalInput").ap()
    wwT = nc.dram_tensor("wwT", [CB, P, DIM], _f32r, kind="ExternalInput").ap()
    btb = nc.dram_tensor("btb", [P, DIM], _f32, kind="ExternalInput").ap()
    bpb = nc.dram_tensor("bpb", [P, DIM], _f32, kind="ExternalInput").ap()
    bgc = nc.dram_tensor("bgc", [P, CB], _f32, kind="ExternalInput").ap()
    offc = nc.dram_tensor("offc", [P, CB], _f32, kind="ExternalInput").ap()
    out4 = nc.dram_tensor("out4", [BL, CB, P, N], _f32, kind="ExternalOutput").ap()

    with TileContext(nc) as tc:
        with tc.tile_pool(name="const", bufs=1) as cpool, \
             tc.tile_pool(name="xin", bufs=2) as xpool, \
             tc.tile_pool(name="work", bufs=4) as wpool, \
             tc.tile_pool(name="att", bufs=2) as apool, \
             tc.tile_pool(name="psum", bufs=6, space="PSUM") as psum:

            wg_sb = cpool.tile([P, CB, DIM], _f32r, tag="wg")
            wt_sb = cpool.tile([P, CB, DIM], _f32r, tag="wt")
            wp_sb = cpool.tile([P, CB, DIM], _f32r, tag="wp")
            ww_sb = cpool.tile([P, CB, DIM], _f32r, tag="ww")
            for cb in range(CB):
                nc.sync.dma_start(wg_sb[:, cb], wgT[cb])
                nc.sync.dma_start(wt_sb[:, cb], wtT[cb])
                nc.sync.dma_start(wp_sb[:, cb], wpT[cb])
                nc.sync.dma_start(ww_sb[:, cb], wwT[cb])
            btb_sb = cpool.tile([P, DIM], _f32, tag="btb")
            bpb_sb = cpool.tile([P, DIM], _f32, tag="bpb")
            bgc_sb = cpool.tile([P, CB], _f32, tag="bgc")
            offc_sb = cpool.tile([P, CB], _f32, tag="offc")
            nc.sync.dma_start(btb_sb, btb)
            nc.sync.dma_start(bpb_sb, bpb)
            nc.sync.dma_start(bgc_sb, bgc)
            nc.sync.dma_start(offc_sb, offc)

            for s in range(BL):
                x_sb = xpool.tile([P, CB, N], _f32r, tag="x")
                xh_sb = xpool.tile([P, CB, N], _f32r, tag="xh")
                for cb in range(CB):
                    nc.sync.dma_start(x_sb[:, cb], x4[s, cb])
                    nc.sync.dma_start(xh_sb[:, cb], xh4[s, cb])

                # thetaT[n, i] (with bt and the 1/dim scale folded in)
                thetaT = wpool.tile([P, NB, DIM], _f32r, tag="work")
                for nb in range(NB):
                    ps = psum.tile([P, DIM], _f32, tag="ps")
                    for cb in range(CB):
                        nc.tensor.matmul(
                            ps, x_sb[:, cb, nb * P:(nb + 1) * P], wt_sb[:, cb],
                            start=(cb == 0), stop=(cb == CB - 1))
                    nc.vector.tensor_tensor(thetaT[:, nb], ps, btb_sb, _add)

                # phiT[n, j]
                phiT = wpool.tile([P, NB, DIM], _f32r, tag="work")
                for nb in range(NB):
                    ps = psum.tile([P, DIM], _f32, tag="ps")
                    for cb in range(CB):
                        nc.tensor.matmul(
                            ps, xh_sb[:, cb, nb * P:(nb + 1) * P], wp_sb[:, cb],
                            start=(cb == 0), stop=(cb == CB - 1))
                    nc.vector.tensor_tensor(phiT[:, nb], ps, bpb_sb, _add)

                # g[o, n]
                g_sb = wpool.tile([P, CB, N], _f32r, tag="work")
                for ob in range(CB):
                    for ch in range(NCH):
                        ps = psum.tile([P, DIM], _f32, tag="ps", name="psg")[:, :CHW]
                        for cb in range(CB):
                            nc.tensor.matmul(
                                ps, wg_sb[:, cb, ob * P:(ob + 1) * P],
                                xh_sb[:, cb, ch * CHW:(ch + 1) * CHW],
                                start=(cb == 0), stop=(cb == CB - 1))
                        nc.vector.tensor_scalar_add(
                            g_sb[:, ob, ch * CHW:(ch + 1) * CHW], ps,
                            bgc_sb[:, ob:ob + 1])

                # attT[j, i] = phiT^T @ thetaT  (contract over n)
                attT = apool.tile([P, CB, DIM], _f32r, tag="att")
                for jb in range(CB):
                    ps = psum.tile([P, DIM], _f32, tag="ps")
                    for nb in range(NB):
                        nc.tensor.matmul(
                            ps, phiT[:, nb, jb * P:(jb + 1) * P], thetaT[:, nb],
                            start=(nb == 0), stop=(nb == NB - 1))
                    nc.any.tensor_copy(out=attT[:, jb], in_=ps)

                # y[i, n] = att @ g
                y_sb = wpool.tile([P, CB, N], _f32r, tag="work")
                for ib in range(CB):
                    for ch in range(NCH):
                        ps = psum.tile([P, DIM], _f32, tag="ps", name="psg")[:, :CHW]
                        for jb in range(CB):
                            nc.tensor.matmul(
                                ps, attT[:, jb, ib * P:(ib + 1) * P],
                                g_sb[:, jb, ch * CHW:(ch + 1) * CHW],
                                start=(jb == 0), stop=(jb == CB - 1))
                        nc.any.tensor_copy(
                            out=y_sb[:, ib, ch * CHW:(ch + 1) * CHW], in_=ps)

                # out[o, n] = (Ww@y)*inv + off + x   (inv folded into wwT)
                o_sb = wpool.tile([P, CB, N], _f32, tag="work")
                for ob in range(CB):
                    for ch in range(NCH):
                        ps = psum.tile([P, DIM], _f32, tag="ps", name="psg")[:, :CHW]
                        for ib in range(CB):
                            nc.tensor.matmul(
                                ps, ww_sb[:, ib, ob * P:(ob + 1) * P],
                                y_sb[:, ib, ch * CHW:(ch + 1) * CHW],
                                start=(ib == 0), stop=(ib == CB - 1))
                        nc.vector.scalar_tensor_tensor(
                            o_sb[:, ob, ch * CHW:(ch + 1) * CHW], ps,
                            offc_sb[:, ob:ob + 1],
                            x_sb[:, ob, ch * CHW:(ch + 1) * CHW].bitcast(_f32),
                            _add, _add)
                for cb in range(CB):
                    nc.sync.dma_start(out4[s, cb], o_sb[:, cb])

    nc.finalize()
    return nc


def _get_program():
    global _PROGRAM
    if _PROGRAM is None:
        _PROGRAM = _build_program()
    return _PROGRAM


def _prep_inputs(x, x_h, Wg, bg, Wt, bt, Wp, bp, Ww, bw, gamma, beta,
                 run_mean, run_var):
    inv = (gamma / np.sqrt(run_var + 1e-5)).astype(np.float32)
    off = ((bw - run_mean) * inv + beta).astype(np.float32)

    xr = np.ascontiguousarray(x.reshape(B, CB, P, N), dtype=np.float32)
    xhr = np.ascontiguousarray(x_h.reshape(B, CB, P, N), dtype=np.float32)

    scale_t = np.float32(1.0 / DIM)
    wgT = np.ascontiguousarray(Wg.T).reshape(CB, P, DIM).astype(np.float32)
    wtT = (np.ascontiguousarray(Wt.T) * scale_t).reshape(CB, P, DIM).astype(np.float32)
    wpT = np.ascontiguousarray(Wp.T).reshape(CB, P, DIM).astype(np.float32)
    wwT = np.ascontiguousarray(Ww.T * inv[None, :]).reshape(CB, P, DIM).astype(np.float32)

    btb = np.tile((bt * scale_t)[None, :], (P, 1)).astype(np.float32)
    bpb = np.tile(bp[None, :], (P, 1)).astype(np.float32)
    bgc = np.ascontiguousarray(bg.reshape(CB, P).T).astype(np.float32)
    offc = np.ascontiguousarray(off.reshape(CB, P).T).astype(np.float32)

    shared = dict(wgT=wgT, wtT=wtT, wpT=wpT, wwT=wwT,
                  btb=btb, bpb=bpb, bgc=bgc, offc=offc)
    in_maps = []
    for k in range(NCORES):
        m = dict(shared)
        m["x4"] = np.ascontiguousarray(xr[k * BL:(k + 1) * BL])
        m["xh4"] = np.ascontiguousarray(xhr[k * BL:(k + 1) * BL])
        in_maps.append(m)
    return in_maps


def run(inputs, trace=False, tmpdir=None):
    nc = _get_program()
    in_maps = _prep_inputs(**inputs)
    res = bass_utils.run_bass_kernel_spmd(
        nc, in_maps, core_ids=list(range(NCORES)), trace=trace, tmpdir=tmpdir)
    outs = [r["out4"] for r in res.results]
    out = np.concatenate(outs, axis=0).reshape(B, DIM, H, W)
    return out.astype(np.float32), res


def kernel(**inputs) -> np.ndarray:
    out, _ = run(inputs)
    return out
